# revision 11
# baseline (speedup 1.0000x reference)
"""Trainium2 Bass kernel for nn_Attention_7670811590880.

Multi-head attention prefill (B=1, S=2048, D=4096, H=32, KVH=8, HD=128),
tensor-parallel over heads across 8 NeuronCores. Core c takes query heads
{c, c+8, c+16, c+24} and kv head c (reference maps q head g -> kv head g%8);
each core emits a bf16 partial [S, D] through its wo rows; the host sums the
8 partials in f32 (halves the output DMA vs f32 partials).

Design (vs the f32r baseline): all matmuls run in bf16 (same 1 col/cycle
PE rate as f32r but no 256-col minimum, 1.0-rate transposes, half the SBUF/
DMA traffic; empirically rel-err ~5e-3 vs the 2e-2 gate):

  Phase A  QKV projections per 512-wide s-block: one pass over 32 d-chunks
           with 6 PSUM banks (4 q heads, k, v) so each streamed x tile feeds
           6 matmuls (keeps PE ahead of the xt DMA rate and the p-state hot);
           RoPE on DVE (evens|odds host permutation trick) -> qT/kT bf16;
           v PE-transposed to v_nat [k,129] whose col 128 is ones.
  Scores   exact-causal: for k-chunk kc, one matmul per <=512-col q-range
           [128*kc, 2048) -> exp on ACT -> e2 (kc,m) bf16 tiles; diagonal
           128-col slice masked by a triangular bf16 multiply on DVE.
           Chunks for heads 0/1 with m<=2 are sprinkled one-per-d-chunk
           into phase A's matmul stream (single-buffer PSUM tile) so ACT exp
           overlaps the PE-bound projection phase; remaining chunks are
           sprinkled between AV rounds in B, gated on e2 buffer recycling.
  AV       ps_o[q,129] += e2(kc)^T @ v_nat(kc): column 128 accumulates the
           softmax denominator Z for free (vs the baseline's extra 1-row
           matmuls, -82k PE cycles/core). Normalize = DVE reciprocal of col
           128 + per-partition scalar multiply -> o_nat bf16 -> PE transpose
           -> oT_all. AV rounds for head h interleave (program order) with
           score emission for head h+2 to keep every engine streaming.
  Phase C  out[st,nt] accumulates the 4 heads' oT^T @ wo, interleaved with
           head 3's AV rounds (C(st) emitted right after norm(h3, st)).
"""
import math
from contextlib import ExitStack

import numpy as np
import ml_dtypes

import concourse.bass as bass
import concourse.tile as tile
from concourse import bacc, mybir
from concourse.bass import ds, ts
from concourse.bass_utils import run_bass_kernel_spmd
from concourse.masks import make_identity

P = 128
SB = 512
F32 = mybir.dt.float32
BF16 = mybir.dt.bfloat16

B, S, D = 1, 2048, 4096
H, KVH, HD = 32, 8, 128
NCORES = 8
NQH = H // NCORES          # 4 query heads per core
DC = D // P                # 32 d-chunks
NB = S // SB               # 4 s-blocks
NT = S // P                # 16 q-subtiles / k-chunks
ROPE_HALF = HD // 2

INV_SQRT_HD = 1.0 / math.sqrt(HD)


def _chunks_of(kc):
    """Score chunks (m, q0, cols) for k-chunk kc: q in [128*kc, 2048)."""
    out = []
    for m in range(kc // 4, 4):
        q0 = max(P * kc, SB * m)
        cols = SB * (m + 1) - q0
        if cols > 0:
            out.append((m, q0, cols))
    return out


def build_attention_kernel(loop_reps=None):
    nc = bacc.Bacc("TRN2", target_bir_lowering=False, debug=False,
                   num_devices=NCORES)
    xT = nc.dram_tensor("xT", [D, S], BF16, kind="ExternalInput").ap()
    wq = nc.dram_tensor("wq", [D, NQH * P], BF16, kind="ExternalInput").ap()
    wk = nc.dram_tensor("wk", [D, P], BF16, kind="ExternalInput").ap()
    wv = nc.dram_tensor("wv", [D, P], BF16, kind="ExternalInput").ap()
    wo = nc.dram_tensor("wo", [NQH * P, D], BF16, kind="ExternalInput").ap()
    cosT = nc.dram_tensor("cosT", [P, S], F32, kind="ExternalInput").ap()
    sinT = nc.dram_tensor("sinT", [P, S], F32, kind="ExternalInput").ap()
    out = nc.dram_tensor("out", [S, D], BF16, kind="ExternalOutput").ap()

    with tile.TileContext(nc) as tc, ExitStack() as top:
        persist = top.enter_context(tc.tile_pool(name="persist", bufs=1))
        e2p = top.enter_context(tc.tile_pool(name="e2p", bufs=2))

        def body():
            with ExitStack() as ctx:
                qT_all = persist.tile([P, NQH, S], BF16, tag="qT", name="qT_all")
                kT_all = persist.tile([P, S], BF16, tag="kT", name="kT_all")
                v_nat = persist.tile([P, NT, P + 1], BF16, tag="vn", name="v_nat")
                oT_all = persist.tile([P, NQH, S], BF16, tag="oT", name="oT_all")
                cc = persist.tile([P, S], F32, tag="cc", name="cc")
                ss = persist.tile([P, S], F32, tag="ssb", name="ss")
                identf = persist.tile([P, P], F32, tag="idf", name="identf")
                ident_bf = persist.tile([P, P], BF16, tag="idb", name="ident_bf")
                tri = persist.tile([P, P], BF16, tag="tri", name="tri")

                nc.sync.dma_start(cc[:], cosT[:, :])
                nc.sync.dma_start(ss[:], sinT[:, :])
                make_identity(nc, identf[:])
                nc.vector.tensor_copy(ident_bf[:], identf[:])
                # tri[p, j] = 1 if j >= p else 0  (keep k<=q on diagonal tiles)
                nc.vector.memset(tri[:], 1.0)
                nc.gpsimd.affine_select(
                    tri[:], tri[:], pattern=[[1, P]],
                    compare_op=mybir.AluOpType.is_ge, fill=0.0,
                    base=0, channel_multiplier=-1)
                nc.vector.memset(v_nat[:, :, P:P + 1], 1.0)

                e2_tiles = {}

                def escore(h, kc, m, q0, cols, spool_, sbufs):
                    ps = spool_.tile([P, SB], F32, tag="ss", name="ps_s",
                                     bufs=sbufs)
                    nc.tensor.matmul(
                        ps[:, 0:cols], kT_all[:, ts(kc, P)],
                        qT_all[:, h, ds(q0, cols)], start=True, stop=True)
                    e = e2p.tile([P, cols], BF16, tag=f"e{kc}_{m}",
                                 name=f"e2_{kc}_{m}",
                                 bufs=(3 if m == 0 else 2))
                    nc.scalar.activation(
                        e[:], ps[:, 0:cols],
                        mybir.ActivationFunctionType.Exp, scale=INV_SQRT_HD)
                    if m == kc // 4:
                        nc.vector.tensor_tensor(
                            e[:, 0:P], e[:, 0:P], tri[:],
                            mybir.AluOpType.mult)
                    e2_tiles[(h, kc, m)] = e

                def escore_batch(h, pairs, spool_, sbufs=2):
                    for kc, m in pairs:
                        for (mm, q0, cols) in _chunks_of(kc):
                            if mm == m:
                                escore(h, kc, m, q0, cols, spool_, sbufs)

                # ---------------- Phase A ----------------
                with ExitStack() as actx:
                    wpool = actx.enter_context(tc.tile_pool(name="wpool", bufs=1))
                    xpool = actx.enter_context(tc.tile_pool(name="xpool", bufs=3))
                    spool = actx.enter_context(tc.tile_pool(name="spool", bufs=2))
                    psA = actx.enter_context(
                        tc.tile_pool(name="psA", bufs=1, space="PSUM"))
                    psSA = actx.enter_context(
                        tc.tile_pool(name="psSA", bufs=1, space="PSUM"))
                    psT = actx.enter_context(
                        tc.tile_pool(name="psT", bufs=1, space="PSUM"))

                    wq_sb = wpool.tile([P, DC, NQH * P], BF16, tag="wq", name="wq_sb")
                    wk_sb = wpool.tile([P, DC, P], BF16, tag="wk", name="wk_sb")
                    wv_sb = wpool.tile([P, DC, P], BF16, tag="wv", name="wv_sb")
                    nc.sync.dma_start(wq_sb[:], wq.rearrange("(o p) m -> p o m", p=P))
                    nc.sync.dma_start(wk_sb[:], wk.rearrange("(o p) m -> p o m", p=P))
                    nc.sync.dma_start(wv_sb[:], wv.rearrange("(o p) m -> p o m", p=P))

                    # early-score chunks for heads 0/1, sprinkled into the
                    # NEXT block's dc loop so exp (ACT) overlaps A compute
                    def early_batch(sb):
                        if sb == 1:
                            return [(h, kc, 0) for h in (0, 1)
                                    for kc in range(4)]
                        if sb == 2:
                            return [(h, kc, 1) for h in (0, 1)
                                    for kc in range(8)]
                        if sb == 3:
                            return [(h, kc, 2) for h in (0, 1)
                                    for kc in range(12)]
                        return []

                    for sb in range(NB):
                        ssl = ds(sb * SB, SB)
                        stg = spool.tile([P, NQH + 1, SB], F32, tag="stg",
                                         name="stg", bufs=1)
                        v_bf = spool.tile([P, SB], BF16, tag="vbf", name="v_bf")
                        early = early_batch(sb)
                        nearly = len(early)
                        pa = [psA.tile([P, SB], F32, tag=f"pa{i}",
                                       name=f"ps_a{i}") for i in range(6)]
                        for dc in range(DC):
                            xt = xpool.tile([P, SB], BF16, tag="xt", name="xt")
                            nc.sync.dma_start(xt[:], xT[ds(dc * P, P), ssl])
                            lhs = [wq_sb[:, dc, ts(0, P)],
                                   wq_sb[:, dc, ts(1, P)],
                                   wq_sb[:, dc, ts(2, P)],
                                   wq_sb[:, dc, ts(3, P)],
                                   wk_sb[:, dc, :],
                                   wv_sb[:, dc, :]]
                            for i in range(6):
                                nc.tensor.matmul(
                                    pa[i][:], lhs[i], xt[:],
                                    start=(dc == 0), stop=(dc == DC - 1))
                            # sprinkle one early-score chunk per dc
                            if nearly and early:
                                h_, kc_, m_ = early.pop(0)
                                for (mm, q0, cols) in _chunks_of(kc_):
                                    if mm == m_:
                                        escore(h_, kc_, m_, q0, cols, psSA, 1)
                        for h_, kc_, m_ in early:
                            for (mm, q0, cols) in _chunks_of(kc_):
                                if mm == m_:
                                    escore(h_, kc_, m_, q0, cols, psSA, 1)

                        nc.scalar.copy(stg[:, 0, :], pa[0][:])
                        nc.scalar.copy(stg[:, 1, :], pa[1][:])
                        nc.vector.tensor_copy(stg[:, 2, :], pa[2][:])
                        nc.vector.tensor_copy(stg[:, 3, :], pa[3][:])
                        nc.scalar.copy(stg[:, 4, :], pa[4][:])
                        nc.scalar.copy(v_bf[:], pa[5][:])

                        # RoPE on DVE: dst = src*cc + rot(src)*ss  (bf16 out)
                        for hh in range(NQH + 1):
                            src = stg[:, hh, :]
                            dst = (kT_all[:, ssl] if hh == NQH
                                   else qT_all[:, hh, ssl])
                            rot = spool.tile([P, SB], F32, tag="rot", name="rot")
                            nc.vector.tensor_copy(
                                rot[0:ROPE_HALF, :], src[ROPE_HALF:P, :])
                            nc.vector.tensor_copy(
                                rot[ROPE_HALF:P, :], src[0:ROPE_HALF, :])
                            tm = spool.tile([P, SB], F32, tag="tm", name="tm")
                            nc.vector.tensor_mul(tm[:], rot[:], ss[:, ssl])
                            nc.vector.tensor_mul(dst, src, cc[:, ssl])
                            nc.vector.tensor_tensor(
                                dst, dst, tm[:], mybir.AluOpType.add)

                        # v -> v_nat subtiles (PE transpose, bf16)
                        for st in range(SB // P):
                            pt = psT.tile([P, P], BF16, tag="tt", name="ps_t",
                                          padded_shape=[P, 4 * P])
                            nc.tensor.transpose(
                                pt[:], v_bf[:, ts(st, P)], ident_bf[:])
                            nc.vector.tensor_copy(
                                v_nat[:, sb * (SB // P) + st, 0:P], pt[:])

                # ---------------- Phase B + C ----------------
                with ExitStack() as bctx:
                    wopool = bctx.enter_context(tc.tile_pool(name="wopool", bufs=1))
                    psSB = bctx.enter_context(
                        tc.tile_pool(name="psSB", bufs=2, space="PSUM"))
                    psT = bctx.enter_context(
                        tc.tile_pool(name="psT", bufs=2, space="PSUM"))
                    psO = bctx.enter_context(
                        tc.tile_pool(name="psO", bufs=2, space="PSUM"))
                    psC = bctx.enter_context(
                        tc.tile_pool(name="psC", bufs=2, space="PSUM"))
                    npool = bctx.enter_context(tc.tile_pool(name="npool", bufs=3))
                    opool = bctx.enter_context(tc.tile_pool(name="opool", bufs=4))

                    wo_sb = wopool.tile([P, NQH, D], BF16, tag="wo", name="wo_sb")
                    nc.sync.dma_start(wo_sb[:], wo.rearrange("(o p) m -> p o m", p=P))

                    pending_tp = []

                    def emit_av(h, t):
                        ps_o = psO.tile([P, P + 1], F32, tag="po", name="ps_o",
                                        padded_shape=[P, SB])
                        m = t // 4
                        for kc in range(t + 1):
                            e = e2_tiles[(h, kc, m)]
                            q0 = max(P * kc, SB * m)
                            off = P * t - q0
                            nc.tensor.matmul(
                                ps_o[:, 0:P + 1], e[:, ds(off, P)],
                                v_nat[:, kc, :], start=(kc == 0), stop=(kc == t))
                        zr = npool.tile([P, 1], F32, tag="zr", name="zr")
                        nc.vector.reciprocal(zr[:], ps_o[:, P:P + 1])
                        onat = npool.tile([P, P], BF16, tag="on", name="onat")
                        nc.vector.tensor_scalar_mul(onat[:], ps_o[:, 0:P], zr[:])
                        pending_tp.append((h, t, onat))

                    def flush_tp():
                        while pending_tp:
                            h, t, onat = pending_tp.pop(0)
                            pt = psT.tile([P, P], BF16, tag="tt", name="ps_t",
                                          padded_shape=[P, 8 * P])
                            nc.tensor.transpose(pt[:], onat[:], ident_bf[:])
                            nc.vector.tensor_copy(
                                oT_all[:, h, ts(t, P)], pt[:])

                    def emit_c(st):
                        for nt in range(D // SB):
                            ps_c = psC.tile([P, SB], F32, tag="pc", name="ps_c")
                            for hh in range(NQH):
                                nc.tensor.matmul(
                                    ps_c[:], oT_all[:, hh, ts(st, P)],
                                    wo_sb[:, hh, ts(nt, SB)],
                                    start=(hh == 0), stop=(hh == NQH - 1))
                            ot = opool.tile([P, SB], BF16, tag="ot", name="ot")
                            if nt % 2 == 0:
                                nc.vector.tensor_copy(ot[:], ps_c[:])
                            else:
                                nc.scalar.copy(ot[:], ps_c[:])
                            nc.sync.dma_start(
                                out[ds(st * P, P), ds(nt * SB, SB)], ot[:])

                    # Pending score chunks per AV stage, each gated on the
                    # round that frees its e2 buffer (bufs=2 per (kc,m) tag).
                    def chunk_list(h, ms, gate_by_m):
                        out = []
                        for m in ms:
                            for kc in range(4 * m + 4):
                                gate = (4 * m + 4) if gate_by_m else 0
                                out.append((h, kc, m, gate))
                        return out

                    def sprinkle(pending, t, per_round):
                        n = 0
                        i = 0
                        while i < len(pending) and n < per_round:
                            if pending[i][3] <= t:
                                h_, kc_, m_, _ = pending.pop(i)
                                for (mm, q0, cols) in _chunks_of(kc_):
                                    if mm == m_:
                                        escore(h_, kc_, m_, q0, cols, psSB, 2)
                                n += 1
                            else:
                                i += 1

                    # h0's m=3 chunks (its m<=2 ran during phase A)
                    escore_batch(0, [(kc, 3) for kc in range(16)], psSB)

                    stage_pending = {
                        0: ([(1, kc, 3, 0) for kc in range(16)]
                            + [(2, kc, 0, 0) for kc in range(4)]
                            + chunk_list(2, (1, 2), True)
                            + [(3, kc, 0, 4) for kc in range(4)]),
                        1: ([(2, kc, 3, 0) for kc in range(16)]
                            + chunk_list(3, (1, 2), True)),
                        2: [(3, kc, 3, 0) for kc in range(16)],
                        3: [],
                    }
                    # interleave ungated/gated for smoother ACT feed
                    def mix(lst):
                        ung = [c for c in lst if c[3] == 0]
                        gat = [c for c in lst if c[3] > 0]
                        out = []
                        while ung or gat:
                            if ung:
                                out.append(ung.pop(0))
                            if gat:
                                out.append(gat.pop(0))
                        return out
                    stage_pending = {k: mix(v) for k, v in stage_pending.items()}

                    for hav in range(NQH):
                        pending = stage_pending[hav]
                        per_round = max(1, (len(pending) + NT - 1) // NT)
                        for t in range(NT):
                            emit_av(hav, t)
                            if len(pending_tp) > 1:
                                h_, t_, onat_ = pending_tp.pop(0)
                                pt = psT.tile([P, P], BF16, tag="tt",
                                              name="ps_t",
                                              padded_shape=[P, 8 * P])
                                nc.tensor.transpose(pt[:], onat_[:], ident_bf[:])
                                nc.vector.tensor_copy(
                                    oT_all[:, h_, ts(t_, P)], pt[:])
                            sprinkle(pending, t, per_round)
                            if hav == 3:
                                flush_tp()
                                if t > 0:
                                    emit_c(t - 1)
                        while pending:
                            sprinkle(pending, NT, 4)
                        if hav < 3:
                            flush_tp()
                    emit_c(NT - 1)

        if loop_reps is not None:
            with tc.For_i(0, loop_reps, 1):
                body()
        else:
            body()

    nc.compile()
    return nc


_ROPE_PERM = np.concatenate([np.arange(0, HD, 2), np.arange(1, HD, 2)])


def shard_inputs(x, wq, wk, wv, wo, freqs_cos, freqs_sin):
    """Host-side sharding/layout. Returns list of 8 per-core input dicts."""
    bf = ml_dtypes.bfloat16
    x2 = np.asarray(x, dtype=np.float32).reshape(S, D)
    xTh = np.ascontiguousarray(x2.T.astype(bf))                   # [D, S] bf16
    cos_h = np.asarray(freqs_cos, np.float32).T                   # [64, S]
    sin_h = np.asarray(freqs_sin, np.float32).T
    cosT = np.ascontiguousarray(np.concatenate([cos_h, cos_h], axis=0))
    sinT = np.ascontiguousarray(np.concatenate([-sin_h, sin_h], axis=0))
    wq = np.asarray(wq, np.float32)
    wk = np.asarray(wk, np.float32)
    wv = np.asarray(wv, np.float32)
    wo = np.asarray(wo, np.float32)
    in_maps = []
    for c in range(NCORES):
        heads = [c + NCORES * r for r in range(NQH)]              # g % 8 == c
        wq_c = np.concatenate(
            [wq[:, g * HD + _ROPE_PERM] for g in heads], axis=1)
        wk_c = wk[:, c * HD + _ROPE_PERM]
        wv_c = wv[:, c * HD:(c + 1) * HD]
        wo_c = np.concatenate([wo[g * HD:(g + 1) * HD, :] for g in heads],
                              axis=0)
        in_maps.append({
            "xT": xTh,
            "wq": np.ascontiguousarray(wq_c.astype(bf)),
            "wk": np.ascontiguousarray(wk_c.astype(bf)),
            "wv": np.ascontiguousarray(wv_c.astype(bf)),
            "wo": np.ascontiguousarray(wo_c.astype(bf)),
            "cosT": cosT,
            "sinT": sinT,
        })
    return in_maps


_NC_CACHE = {}


def _get_nc():
    if "nc" not in _NC_CACHE:
        _NC_CACHE["nc"] = build_attention_kernel()
    return _NC_CACHE["nc"]


def kernel(x, wq, wk, wv, wo, freqs_cos, freqs_sin, mask, cache_k, cache_v,
           start_pos):
    assert int(start_pos) == 0, "kernel assumes prefill at start_pos=0"
    in_maps = shard_inputs(x, wq, wk, wv, wo, freqs_cos, freqs_sin)
    nc = _get_nc()
    res = run_bass_kernel_spmd(nc, in_maps, core_ids=list(range(NCORES)))
    acc = np.zeros((S, D), np.float32)
    for c in range(NCORES):
        acc += res.results[c]["out"].astype(np.float32)
    return acc.reshape(B, S, D)


# revision 12
# speedup vs baseline: 1.2360x; 1.2360x over previous
"""Trainium2 Bass kernel for nn_Attention_7670811590880.

Multi-head attention prefill (B=1, S=2048, D=4096, H=32, KVH=8, HD=128),
tensor-parallel over heads across 8 NeuronCores. Core c takes query heads
{c, c+8, c+16, c+24} and kv head c (reference maps q head g -> kv head g%8);
each core emits a bf16 partial [S, D] through its wo rows; the host sums the
8 partials in f32 (halves the output DMA vs f32 partials).

Design (vs the f32r baseline): all matmuls run in bf16 (same 1 col/cycle
PE rate as f32r but no 256-col minimum, 1.0-rate transposes, half the SBUF/
DMA traffic; empirically rel-err ~5e-3 vs the 2e-2 gate):

  Phase A  QKV projections per 512-wide s-block: one pass over 32 d-chunks
           with 6 PSUM banks (4 q heads, k, v) so each streamed x tile feeds
           6 matmuls (keeps PE ahead of the xt DMA rate and the p-state hot);
           RoPE on DVE (evens|odds host permutation trick) -> qT/kT bf16;
           v PE-transposed to v_nat [k,129] whose col 128 is ones.
  Scores   exact-causal: for k-chunk kc, one matmul per <=512-col q-range
           [128*kc, 2048) -> exp on ACT -> e2 (kc,m) bf16 tiles; diagonal
           128-col slice masked by a triangular bf16 multiply on DVE.
           Chunks for heads 0/1 with m<=2 are sprinkled one-per-d-chunk
           into phase A's matmul stream (single-buffer PSUM tile) so ACT exp
           overlaps the PE-bound projection phase; remaining chunks are
           sprinkled between AV rounds in B, gated on e2 buffer recycling.
  AV       ps_o[q,129] += e2(kc)^T @ v_nat(kc): column 128 accumulates the
           softmax denominator Z for free (vs the baseline's extra 1-row
           matmuls, -82k PE cycles/core). Normalize = DVE reciprocal of col
           128 + per-partition scalar multiply -> o_nat bf16 -> PE transpose
           -> oT_all. AV rounds for head h interleave (program order) with
           score emission for head h+2 to keep every engine streaming.
  Phase C  out[st,nt] accumulates the 4 heads' oT^T @ wo, interleaved with
           head 3's AV rounds (C(st) emitted right after norm(h3, st)).
"""
import math
from contextlib import ExitStack

import numpy as np
import ml_dtypes

import concourse.bass as bass
import concourse.tile as tile
from concourse import bacc, mybir
from concourse.bass import ds, ts
from concourse.bass_utils import run_bass_kernel_spmd
from concourse.masks import make_identity

P = 128
SB = 512
F32 = mybir.dt.float32
BF16 = mybir.dt.bfloat16

B, S, D = 1, 2048, 4096
H, KVH, HD = 32, 8, 128
NCORES = 8
NQH = H // NCORES          # 4 query heads per core
DC = D // P                # 32 d-chunks
NB = S // SB               # 4 s-blocks
NT = S // P                # 16 q-subtiles / k-chunks
ROPE_HALF = HD // 2

INV_SQRT_HD = 1.0 / math.sqrt(HD)


def _chunks_of(kc):
    """Score chunks (m, q0, cols) for k-chunk kc: q in [128*kc, 2048)."""
    out = []
    for m in range(kc // 4, 4):
        q0 = max(P * kc, SB * m)
        cols = SB * (m + 1) - q0
        if cols > 0:
            out.append((m, q0, cols))
    return out


def build_attention_kernel(loop_reps=None):
    nc = bacc.Bacc("TRN2", target_bir_lowering=False, debug=False,
                   num_devices=NCORES)
    xT = nc.dram_tensor("xT", [D, S], BF16, kind="ExternalInput").ap()
    wq = nc.dram_tensor("wq", [D, NQH * P], BF16, kind="ExternalInput").ap()
    wk = nc.dram_tensor("wk", [D, P], BF16, kind="ExternalInput").ap()
    wv = nc.dram_tensor("wv", [D, P], BF16, kind="ExternalInput").ap()
    wo = nc.dram_tensor("wo", [NQH * P, D], BF16, kind="ExternalInput").ap()
    cosT = nc.dram_tensor("cosT", [P, S], F32, kind="ExternalInput").ap()
    sinT = nc.dram_tensor("sinT", [P, S], F32, kind="ExternalInput").ap()
    out = nc.dram_tensor("out", [S, D], BF16, kind="ExternalOutput").ap()

    with tile.TileContext(nc) as tc, ExitStack() as top:
        persist = top.enter_context(tc.tile_pool(name="persist", bufs=1))
        e2p = top.enter_context(tc.tile_pool(name="e2p", bufs=2))

        def body():
            with ExitStack() as ctx:
                qT_all = persist.tile([P, NQH, S], BF16, tag="qT", name="qT_all")
                kT_all = persist.tile([P, S], BF16, tag="kT", name="kT_all")
                v_nat = persist.tile([P, NT, P + 1], BF16, tag="vn", name="v_nat")
                oT_all = persist.tile([P, NQH, S], BF16, tag="oT", name="oT_all")
                cc = persist.tile([P, S], F32, tag="cc", name="cc")
                ss = persist.tile([P, S], F32, tag="ssb", name="ss")
                identf = persist.tile([P, P], F32, tag="idf", name="identf")
                ident_bf = persist.tile([P, P], BF16, tag="idb", name="ident_bf")
                tri = persist.tile([P, P], BF16, tag="tri", name="tri")

                nc.sync.dma_start(cc[:], cosT[:, :])
                nc.sync.dma_start(ss[:], sinT[:, :])
                make_identity(nc, identf[:])
                nc.vector.tensor_copy(ident_bf[:], identf[:])
                # tri[p, j] = 1 if j >= p else 0  (keep k<=q on diagonal tiles)
                nc.vector.memset(tri[:], 1.0)
                nc.gpsimd.affine_select(
                    tri[:], tri[:], pattern=[[1, P]],
                    compare_op=mybir.AluOpType.is_ge, fill=0.0,
                    base=0, channel_multiplier=-1)
                nc.vector.memset(v_nat[:, :, P:P + 1], 1.0)

                e2_tiles = {}

                def escore(h, kc, m, q0, cols, spool_, sbufs):
                    ps = spool_.tile([P, SB], F32, tag="ss", name="ps_s",
                                     bufs=sbufs)
                    nc.tensor.matmul(
                        ps[:, 0:cols], kT_all[:, ts(kc, P)],
                        qT_all[:, h, ds(q0, cols)], start=True, stop=True)
                    e = e2p.tile([P, cols], BF16, tag=f"e{kc}_{m}",
                                 name=f"e2_{kc}_{m}")
                    nc.scalar.activation(
                        e[:], ps[:, 0:cols],
                        mybir.ActivationFunctionType.Exp, scale=INV_SQRT_HD)
                    if m == kc // 4:
                        nc.vector.tensor_tensor(
                            e[:, 0:P], e[:, 0:P], tri[:],
                            mybir.AluOpType.mult)
                    e2_tiles[(h, kc, m)] = e

                def escore_batch(h, pairs, spool_, sbufs=2):
                    for kc, m in pairs:
                        for (mm, q0, cols) in _chunks_of(kc):
                            if mm == m:
                                escore(h, kc, m, q0, cols, spool_, sbufs)

                # ---------------- Phase A ----------------
                with ExitStack() as actx:
                    wpool = actx.enter_context(tc.tile_pool(name="wpool", bufs=1))
                    xpool = actx.enter_context(tc.tile_pool(name="xpool", bufs=3))
                    spool = actx.enter_context(tc.tile_pool(name="spool", bufs=2))
                    psA = actx.enter_context(
                        tc.tile_pool(name="psA", bufs=1, space="PSUM"))
                    psSA = actx.enter_context(
                        tc.tile_pool(name="psSA", bufs=1, space="PSUM"))
                    psT = actx.enter_context(
                        tc.tile_pool(name="psT", bufs=1, space="PSUM"))

                    wq_sb = wpool.tile([P, DC, NQH * P], BF16, tag="wq", name="wq_sb")
                    wk_sb = wpool.tile([P, DC, P], BF16, tag="wk", name="wk_sb")
                    wv_sb = wpool.tile([P, DC, P], BF16, tag="wv", name="wv_sb")
                    nc.sync.dma_start(wq_sb[:], wq.rearrange("(o p) m -> p o m", p=P))
                    nc.sync.dma_start(wk_sb[:], wk.rearrange("(o p) m -> p o m", p=P))
                    nc.sync.dma_start(wv_sb[:], wv.rearrange("(o p) m -> p o m", p=P))

                    # early-score chunks for heads 0/1, sprinkled into the
                    # NEXT block's dc loop so exp (ACT) overlaps A compute
                    def early_batch(sb):
                        if sb == 1:
                            return [(h, kc, 0) for h in (0, 1)
                                    for kc in range(4)]
                        if sb == 2:
                            return [(h, kc, 1) for h in (0, 1)
                                    for kc in range(8)]
                        if sb == 3:
                            return [(h, kc, 2) for h in (0, 1)
                                    for kc in range(12)]
                        return []

                    for sb in range(NB):
                        ssl = ds(sb * SB, SB)
                        stg = spool.tile([P, NQH + 1, SB], F32, tag="stg",
                                         name="stg", bufs=1)
                        v_bf = spool.tile([P, SB], BF16, tag="vbf", name="v_bf")
                        early = early_batch(sb)
                        nearly = len(early)
                        pa = [psA.tile([P, SB], F32, tag=f"pa{i}",
                                       name=f"ps_a{i}") for i in range(6)]
                        for dc in range(DC):
                            xt = xpool.tile([P, SB], BF16, tag="xt", name="xt")
                            nc.sync.dma_start(xt[:], xT[ds(dc * P, P), ssl])
                            lhs = [wq_sb[:, dc, ts(0, P)],
                                   wq_sb[:, dc, ts(1, P)],
                                   wq_sb[:, dc, ts(2, P)],
                                   wq_sb[:, dc, ts(3, P)],
                                   wk_sb[:, dc, :],
                                   wv_sb[:, dc, :]]
                            for i in range(6):
                                nc.tensor.matmul(
                                    pa[i][:], lhs[i], xt[:],
                                    start=(dc == 0), stop=(dc == DC - 1))
                            # sprinkle one early-score chunk per dc
                            if nearly and early:
                                h_, kc_, m_ = early.pop(0)
                                for (mm, q0, cols) in _chunks_of(kc_):
                                    if mm == m_:
                                        escore(h_, kc_, m_, q0, cols, psSA, 1)
                        for h_, kc_, m_ in early:
                            for (mm, q0, cols) in _chunks_of(kc_):
                                if mm == m_:
                                    escore(h_, kc_, m_, q0, cols, psSA, 1)

                        nc.scalar.copy(stg[:, 0, :], pa[0][:])
                        nc.scalar.copy(stg[:, 1, :], pa[1][:])
                        nc.vector.tensor_copy(stg[:, 2, :], pa[2][:])
                        nc.vector.tensor_copy(stg[:, 3, :], pa[3][:])
                        nc.scalar.copy(stg[:, 4, :], pa[4][:])
                        nc.scalar.copy(v_bf[:], pa[5][:])

                        # RoPE on DVE: dst = src*cc + rot(src)*ss  (bf16 out)
                        for hh in range(NQH + 1):
                            src = stg[:, hh, :]
                            dst = (kT_all[:, ssl] if hh == NQH
                                   else qT_all[:, hh, ssl])
                            rot = spool.tile([P, SB], F32, tag="rot", name="rot")
                            nc.vector.tensor_copy(
                                rot[0:ROPE_HALF, :], src[ROPE_HALF:P, :])
                            nc.vector.tensor_copy(
                                rot[ROPE_HALF:P, :], src[0:ROPE_HALF, :])
                            tm = spool.tile([P, SB], F32, tag="tm", name="tm")
                            nc.vector.tensor_mul(tm[:], rot[:], ss[:, ssl])
                            nc.vector.tensor_mul(dst, src, cc[:, ssl])
                            nc.vector.tensor_tensor(
                                dst, dst, tm[:], mybir.AluOpType.add)

                        # v -> v_nat subtiles (PE transpose, bf16)
                        for st in range(SB // P):
                            pt = psT.tile([P, P], BF16, tag="tt", name="ps_t",
                                          padded_shape=[P, 4 * P])
                            nc.tensor.transpose(
                                pt[:], v_bf[:, ts(st, P)], ident_bf[:])
                            nc.vector.tensor_copy(
                                v_nat[:, sb * (SB // P) + st, 0:P], pt[:])

                # ---------------- Phase B + C ----------------
                with ExitStack() as bctx:
                    wopool = bctx.enter_context(tc.tile_pool(name="wopool", bufs=1))
                    psSB = bctx.enter_context(
                        tc.tile_pool(name="psSB", bufs=2, space="PSUM"))
                    psT = bctx.enter_context(
                        tc.tile_pool(name="psT", bufs=2, space="PSUM"))
                    psO = bctx.enter_context(
                        tc.tile_pool(name="psO", bufs=2, space="PSUM"))
                    psC = bctx.enter_context(
                        tc.tile_pool(name="psC", bufs=2, space="PSUM"))
                    npool = bctx.enter_context(tc.tile_pool(name="npool", bufs=3))
                    opool = bctx.enter_context(tc.tile_pool(name="opool", bufs=4))

                    wo_sb = wopool.tile([P, NQH, D], BF16, tag="wo", name="wo_sb")
                    nc.sync.dma_start(wo_sb[:], wo.rearrange("(o p) m -> p o m", p=P))

                    pending_tp = []

                    def emit_av(h, t):
                        ps_o = psO.tile([P, P + 1], F32, tag="po", name="ps_o",
                                        padded_shape=[P, SB])
                        m = t // 4
                        for kc in range(t + 1):
                            e = e2_tiles[(h, kc, m)]
                            q0 = max(P * kc, SB * m)
                            off = P * t - q0
                            nc.tensor.matmul(
                                ps_o[:, 0:P + 1], e[:, ds(off, P)],
                                v_nat[:, kc, :], start=(kc == 0), stop=(kc == t))
                        zr = npool.tile([P, 1], F32, tag="zr", name="zr")
                        nc.vector.reciprocal(zr[:], ps_o[:, P:P + 1])
                        onat = npool.tile([P, P], BF16, tag="on", name="onat")
                        nc.vector.tensor_scalar_mul(onat[:], ps_o[:, 0:P], zr[:])
                        pending_tp.append((h, t, onat))

                    def flush_tp():
                        while pending_tp:
                            h, t, onat = pending_tp.pop(0)
                            pt = psT.tile([P, P], BF16, tag="tt", name="ps_t",
                                          padded_shape=[P, 8 * P])
                            nc.tensor.transpose(pt[:], onat[:], ident_bf[:])
                            nc.vector.tensor_copy(
                                oT_all[:, h, ts(t, P)], pt[:])

                    def emit_c(st):
                        for nt in range(D // SB):
                            ps_c = psC.tile([P, SB], F32, tag="pc", name="ps_c")
                            for hh in range(NQH):
                                nc.tensor.matmul(
                                    ps_c[:], oT_all[:, hh, ts(st, P)],
                                    wo_sb[:, hh, ts(nt, SB)],
                                    start=(hh == 0), stop=(hh == NQH - 1))
                            ot = opool.tile([P, SB], BF16, tag="ot", name="ot")
                            if nt % 2 == 0:
                                nc.vector.tensor_copy(ot[:], ps_c[:])
                            else:
                                nc.scalar.copy(ot[:], ps_c[:])
                            nc.sync.dma_start(
                                out[ds(st * P, P), ds(nt * SB, SB)], ot[:])

                    # Pending score chunks per AV stage, each gated on the
                    # round that frees its e2 buffer (bufs=2 per (kc,m) tag).
                    def chunk_list(h, ms, gate_by_m):
                        out = []
                        for m in ms:
                            for kc in range(4 * m + 4):
                                gate = (4 * m + 4) if gate_by_m else 0
                                out.append((h, kc, m, gate))
                        return out

                    def sprinkle(pending, t, per_round):
                        n = 0
                        i = 0
                        while i < len(pending) and n < per_round:
                            if pending[i][3] <= t:
                                h_, kc_, m_, _ = pending.pop(i)
                                for (mm, q0, cols) in _chunks_of(kc_):
                                    if mm == m_:
                                        escore(h_, kc_, m_, q0, cols, psSB, 2)
                                n += 1
                            else:
                                i += 1

                    # h0's m=3 chunks (its m<=2 ran during phase A)
                    escore_batch(0, [(kc, 3) for kc in range(16)], psSB)

                    stage_pending = {
                        0: ([(1, kc, 3, 0) for kc in range(16)]
                            + chunk_list(2, (0, 1, 2), True)),
                        1: ([(2, kc, 3, 0) for kc in range(16)]
                            + chunk_list(3, (0, 1, 2), True)),
                        2: [(3, kc, 3, 0) for kc in range(16)],
                        3: [],
                    }
                    # interleave ungated/gated for smoother ACT feed
                    def mix(lst):
                        ung = [c for c in lst if c[3] == 0]
                        gat = [c for c in lst if c[3] > 0]
                        out = []
                        while ung or gat:
                            if ung:
                                out.append(ung.pop(0))
                            if gat:
                                out.append(gat.pop(0))
                        return out
                    stage_pending = {k: mix(v) for k, v in stage_pending.items()}

                    for hav in range(NQH):
                        pending = stage_pending[hav]
                        per_round = max(1, (len(pending) + NT - 1) // NT)
                        for t in range(NT):
                            emit_av(hav, t)
                            if len(pending_tp) > 1:
                                h_, t_, onat_ = pending_tp.pop(0)
                                pt = psT.tile([P, P], BF16, tag="tt",
                                              name="ps_t",
                                              padded_shape=[P, 8 * P])
                                nc.tensor.transpose(pt[:], onat_[:], ident_bf[:])
                                nc.vector.tensor_copy(
                                    oT_all[:, h_, ts(t_, P)], pt[:])
                            sprinkle(pending, t, per_round)
                            if hav == 3:
                                flush_tp()
                                if t > 0:
                                    emit_c(t - 1)
                        while pending:
                            sprinkle(pending, NT, 4)
                        if hav < 3:
                            flush_tp()
                    emit_c(NT - 1)

        if loop_reps is not None:
            with tc.For_i(0, loop_reps, 1):
                body()
        else:
            body()

    nc.compile()
    return nc


_ROPE_PERM = np.concatenate([np.arange(0, HD, 2), np.arange(1, HD, 2)])


def shard_inputs(x, wq, wk, wv, wo, freqs_cos, freqs_sin):
    """Host-side sharding/layout. Returns list of 8 per-core input dicts."""
    bf = ml_dtypes.bfloat16
    x2 = np.asarray(x, dtype=np.float32).reshape(S, D)
    xTh = np.ascontiguousarray(x2.T.astype(bf))                   # [D, S] bf16
    cos_h = np.asarray(freqs_cos, np.float32).T                   # [64, S]
    sin_h = np.asarray(freqs_sin, np.float32).T
    cosT = np.ascontiguousarray(np.concatenate([cos_h, cos_h], axis=0))
    sinT = np.ascontiguousarray(np.concatenate([-sin_h, sin_h], axis=0))
    wq = np.asarray(wq, np.float32)
    wk = np.asarray(wk, np.float32)
    wv = np.asarray(wv, np.float32)
    wo = np.asarray(wo, np.float32)
    in_maps = []
    for c in range(NCORES):
        heads = [c + NCORES * r for r in range(NQH)]              # g % 8 == c
        wq_c = np.concatenate(
            [wq[:, g * HD + _ROPE_PERM] for g in heads], axis=1)
        wk_c = wk[:, c * HD + _ROPE_PERM]
        wv_c = wv[:, c * HD:(c + 1) * HD]
        wo_c = np.concatenate([wo[g * HD:(g + 1) * HD, :] for g in heads],
                              axis=0)
        in_maps.append({
            "xT": xTh,
            "wq": np.ascontiguousarray(wq_c.astype(bf)),
            "wk": np.ascontiguousarray(wk_c.astype(bf)),
            "wv": np.ascontiguousarray(wv_c.astype(bf)),
            "wo": np.ascontiguousarray(wo_c.astype(bf)),
            "cosT": cosT,
            "sinT": sinT,
        })
    return in_maps


_NC_CACHE = {}


def _get_nc():
    if "nc" not in _NC_CACHE:
        _NC_CACHE["nc"] = build_attention_kernel()
    return _NC_CACHE["nc"]


def kernel(x, wq, wk, wv, wo, freqs_cos, freqs_sin, mask, cache_k, cache_v,
           start_pos):
    assert int(start_pos) == 0, "kernel assumes prefill at start_pos=0"
    in_maps = shard_inputs(x, wq, wk, wv, wo, freqs_cos, freqs_sin)
    nc = _get_nc()
    res = run_bass_kernel_spmd(nc, in_maps, core_ids=list(range(NCORES)))
    acc = np.zeros((S, D), np.float32)
    for c in range(NCORES):
        acc += res.results[c]["out"].astype(np.float32)
    return acc.reshape(B, S, D)
